# revision 28
# baseline (speedup 1.0000x reference)
"""Trainium2 Bass kernel for causal self-attention (RoPE + per-head RMSNorm).

Reference computation (B=2, T=2048, C=1024, H=16, D=64):
    q = rope(rmsnorm(x @ Wq.T)); k = rope(rmsnorm(x @ Wk.T)); v = x @ Wv.T
    out = softmax(causal(q k^T / sqrt(D))) v @ Wo.T

Sharding over 8 NeuronCores: core c -> batch b = c//4, head-group g = c%4
(4 heads = 256 features per group).  Everything on-chip is computed in a
feature-major ("transposed") layout so no PE transposes are needed:
  - scores are computed as S^T[tk, tq] tiles, softmax runs over the
    partition axis using matmul-with-ones tricks (denominator comes from a
    ones column appended to V), and the final division is applied via a
    gpsimd partition_broadcast + DVE multiply.
  - causal structure is exploited at the instruction level: score /
    exp / attn_v instructions on diagonal tiles only cover the valid
    column window; the triangle mask is a single [128,128] multiply.
  - rope's half-rotation is a DVE stream_shuffle: the host permutes the
    W{q,k} output features so rope partners live in the same 32-partition
    quadrant (scores are invariant to a consistent permutation of d).
  - attention output Y^T (feature-major) is exchanged with an AllToAll
    over all 8 cores using token-slices of T/8 per slot, so every slot
    carries useful data; rank r then runs o_proj for token window
    [r*T/8, (r+1)*T/8) of BOTH batches.  The o_proj is split into an
    hp0 half (runs while the second AllToAll is in flight) and an hp1
    half, accumulating in PSUM across the two.
Host side: shards/permutes/transposes inputs (bf16), assembles fp32 output.
"""

import os
import sys

for _p in ("/opt/trn_rl_repo", "/root/.axon_site/_ro/trn_rl_repo"):
    if os.path.isdir(_p) and _p not in sys.path:
        sys.path.insert(0, _p)

import numpy as np
import ml_dtypes

import concourse.bass as bass
from concourse import bacc
import concourse.tile as tile
import concourse.mybir as mybir

BF16 = mybir.dt.bfloat16
F32 = mybir.dt.float32
AF = mybir.ActivationFunctionType
ALU = mybir.AluOpType

B, T, C, H, D = 2, 2048, 1024, 16, 64
N_CORES = 8
GH = 4  # heads per core
GF = GH * D  # features per core (256)
TB = 512  # token block (matmul N)
KT = C // 128  # 8 contraction k-tiles
EPS = float(np.finfo(np.float32).eps)
ROPE_BASE = 10000.0

# stream_shuffle mask: swap 16-row halves within each 32-partition quadrant
SHUF16 = list(range(16, 32)) + list(range(0, 16))

# bisection switches
NO_PB = bool(int(os.environ.get("V2_NO_PB", "1")))        # matmul broadcasts
SYNC_LOADS = bool(int(os.environ.get("V2_SYNC_LOADS", "0")))  # loads on SP
NO_SHUF = bool(int(os.environ.get("V2_NO_SHUF", "0")))    # pswap matmul rope
DEBUG_DUMP = bool(int(os.environ.get("V2_DEBUG", "0")))   # dump q/k/y intermediates


def build_nc(t=T):
    ntb = t // TB  # tq blocks
    ntt = t // 128  # token 128-tiles
    tsl = t // 8  # per-rank token slice for o_proj (per batch)

    nc = bacc.Bacc("TRN2", target_bir_lowering=False, debug=False, num_devices=N_CORES)

    xt = nc.dram_tensor("xt", [C, t], BF16, kind="ExternalInput")
    wq = nc.dram_tensor("wq", [C, GF], BF16, kind="ExternalInput")
    wk = nc.dram_tensor("wk", [C, GF], BF16, kind="ExternalInput")
    wv = nc.dram_tensor("wv", [C, GF], BF16, kind="ExternalInput")
    wo = nc.dram_tensor("wo", [C, C], BF16, kind="ExternalInput")
    cosf = nc.dram_tensor("cosf", [128, t], BF16, kind="ExternalInput")
    sinf = nc.dram_tensor("sinf", [128, t], BF16, kind="ExternalInput")
    blk2 = nc.dram_tensor("blk2", [128, 2], BF16, kind="ExternalInput")
    maskt = nc.dram_tensor("maskt", [128, 128], BF16, kind="ExternalInput")
    pswap = nc.dram_tensor("pswap", [128, 128], BF16, kind="ExternalInput")
    eqb = nc.dram_tensor("eqb", [2, 128], BF16, kind="ExternalInput")
    ekb = nc.dram_tensor("ekb", [2, 128], BF16, kind="ExternalInput")
    out = nc.dram_tensor("out", [C, 2 * tsl], BF16, kind="ExternalOutput")
    if DEBUG_DUMP:
        qdbg = nc.dram_tensor("qdbg", [2 * 128, t], F32, kind="ExternalOutput")
        kdbg = nc.dram_tensor("kdbg", [2 * 128, t], F32, kind="ExternalOutput")
        ydbg = nc.dram_tensor("ydbg", [2 * 128, t], F32, kind="ExternalOutput")
        ygdbg = nc.dram_tensor("ygdbg", [16 * 128, tsl], F32, kind="ExternalOutput")

    with tile.TileContext(nc) as tc:
        with (
            nc.allow_low_precision(reason="bf16 compute by design"),
            tc.tile_pool(name="p_xt", bufs=KT) as p_xt,
            tc.tile_pool(name="p_w", bufs=KT) as p_w,
            tc.tile_pool(name="p_wo", bufs=KT) as p_wo,
            tc.tile_pool(name="p_tab", bufs=1) as p_tab,
            tc.tile_pool(name="p_qk", bufs=2) as p_qk,
            tc.tile_pool(name="p_v", bufs=ntt) as p_v,
            tc.tile_pool(name="p_y", bufs=2) as p_y,
            tc.tile_pool(name="p_yg", bufs=2) as p_yg,
            tc.tile_pool(name="p_pt", bufs=20) as p_pt,
            tc.tile_pool(name="p_tmp", bufs=2) as p_tmp,
            tc.tile_pool(name="p_osb", bufs=4) as p_osb,
            tc.tile_pool(name="p_mm", bufs=4, space="PSUM") as p_mm,
            tc.tile_pool(name="p_oacc", bufs=4, space="PSUM") as p_oacc,
            tc.tile_pool(name="p_dram", bufs=2, space="DRAM") as p_dram,
        ):
            # ---- load inputs -------------------------------------------------
            # Deps are aggregated counting-semaphore thresholds keyed on
            # emission order, so loads are emitted just-in-time: anything
            # emitted before a PE instruction serializes ahead of it.
            xt_sb = [None] * KT
            wq_sb = [None] * KT
            wk_sb = [None] * KT
            wv_sb = []
            x_t = p_xt.tile([128, t], BF16, tag="xt", name="xt0")
            nc.sync.dma_start(x_t[:], xt[0:128, :])
            xt_sb[0] = x_t
            w_t = p_w.tile([128, GF], BF16, tag="wq")
            nc.sync.dma_start(w_t[:], wq[0:128, :])
            wq_sb[0] = w_t

            def load_qx(ct):
                # just-in-time load of (wq[ct], xt[ct]) pairs
                if ct >= KT or xt_sb[ct] is not None:
                    return
                eng = nc.sync if ct % 2 == 0 else nc.scalar
                w_t = p_w.tile([128, GF], BF16, tag="wq")
                eng.dma_start(w_t[:], wq[ct * 128 : (ct + 1) * 128, :])
                wq_sb[ct] = w_t
                x_t = p_xt.tile([128, t], BF16, tag="xt", name=f"xt{ct}")
                eng.dma_start(x_t[:], xt[ct * 128 : (ct + 1) * 128, :])
                xt_sb[ct] = x_t

            cos_sb = p_tab.tile([128, t], BF16, tag="cos")
            sin_sb = p_tab.tile([128, t], BF16, tag="sin")
            blk2_sb = p_tab.tile([128, 2], BF16, tag="blk2")
            eqb_sb = p_tab.tile([2, 128], BF16, tag="eqb")
            ekb_sb = p_tab.tile([2, 128], BF16, tag="ekb")

            def load_tables():
                nc.scalar.dma_start(cos_sb[:], cosf[:])
                nc.scalar.dma_start(sin_sb[:], sinf[:])
                nc.scalar.dma_start(blk2_sb[:], blk2[:])
                nc.scalar.dma_start(eqb_sb[:], eqb[:])
                nc.scalar.dma_start(ekb_sb[:], ekb[:])
                if NO_SHUF:
                    nc.scalar.dma_start(pswap_sb[:], pswap[:])

            if NO_SHUF:
                pswap_sb = p_tab.tile([128, 128], BF16, tag="pswap")
            if NO_PB:
                ones64 = p_tab.tile([1, 64], BF16, tag="ones64")
                nc.vector.memset(ones64[:], 1.0)
            eps_sb = p_tab.tile([2, 1], F32, tag="eps")
            nc.vector.memset(eps_sb[:], EPS)

            def load_wv():
                for ct in range(KT):
                    w_t = p_w.tile([128, GF], BF16, tag="wv")
                    eng = nc.sync if ct % 2 == 0 else nc.scalar
                    eng.dma_start(w_t[:], wv[ct * 128 : (ct + 1) * 128, :])
                    wv_sb.append(w_t)

            def load_wk():
                for ct in range(KT):
                    w_t = p_w.tile([128, GF], BF16, tag="wk")
                    eng = nc.sync if ct % 2 == 0 else nc.scalar
                    eng.dma_start(w_t[:], wk[ct * 128 : (ct + 1) * 128, :])
                    wk_sb[ct] = w_t

            mask_sb = None
            wo_sb = []

            def load_mask():
                nonlocal mask_sb
                mask_sb = p_tab.tile([128, 128], BF16, tag="mask")
                nc.scalar.dma_start(mask_sb[:], maskt[:])

            def load_wo():
                for mt in range(KT):
                    w_t = p_wo.tile([128, C], BF16, tag="wo")
                    eng = nc.sync if mt % 2 == 0 else nc.scalar
                    eng.dma_start(w_t[:], wo[mt * 128 : (mt + 1) * 128, :])
                    wo_sb.append(w_t)

            # ---- q/k/v projections, rmsnorm + rope (software-pipelined) -----
            qh_sb = [p_qk.tile([128, t], BF16, tag="qk0", name="qh0"),
                     p_qk.tile([128, t], BF16, tag="qk1", name="qh1")]
            kh_sb = [p_qk.tile([128, t], BF16, tag="qk0", name="kh0"),
                     p_qk.tile([128, t], BF16, tag="qk1", name="kh1")]
            v_sb = []

            def emit_chain_tail(pq, pss, dst, jb, eb_sb):
                # skinny rms stats -> broadcast matmul -> rope (Square and
                # the pss matmul were emitted earlier, see emit_proj_group)
                sqm = p_tmp.tile([2, TB], F32, tag="sqm", name="sqm")
                nc.scalar.activation(
                    sqm[:], pss[0:2, :], AF.Sqrt, scale=1.0 / D, bias=eps_sb[:]
                )
                invf = p_tmp.tile([2, TB], F32, tag="invf", name="invf")
                nc.vector.reciprocal_approx_fast(out=invf[:], in_=sqm[:])
                inv = p_tmp.tile([2, TB], BF16, tag="inv", name="inv")
                nc.vector.tensor_copy(inv[:], invf[:])
                pinvb = p_mm.tile([128, TB], F32, tag="mm", name="pinvb")
                nc.tensor.matmul(pinvb[:], eb_sb[:], inv[:], start=True, stop=True)
                invb = p_tmp.tile([128, TB], BF16, tag="invb", name="invb")
                nc.vector.tensor_copy(invb[:], pinvb[:])
                qn = p_tmp.tile([128, TB], BF16, tag="qn", name="qn")
                nc.vector.tensor_mul(qn[:], pq[:], invb[:])
                # rope: dst = qn*cos + swap(qn)*sin_signed
                qsw = p_tmp.tile([128, TB], BF16, tag="qsw", name="qsw")
                if NO_SHUF:
                    pqs = p_mm.tile([128, TB], F32, tag="mm", name="pqs")
                    nc.tensor.matmul(pqs[:], pswap_sb[:], qn[:], start=True, stop=True)
                    nc.vector.tensor_copy(qsw[:], pqs[:])
                else:
                    nc.vector.stream_shuffle(qsw[:], qn[:], SHUF16)
                t1 = p_tmp.tile([128, TB], BF16, tag="t1", name="t1")
                nc.vector.tensor_mul(t1[:], qn[:], cos_sb[:, jb])
                t2 = p_tmp.tile([128, TB], BF16, tag="t2", name="t2")
                nc.vector.tensor_mul(t2[:], qsw[:], sin_sb[:, jb])
                nc.vector.tensor_add(dst[:, jb], t1[:], t2[:])

            def emit_v(tt):
                pv = p_mm.tile([128, TB], F32, tag="mm", name="pv")
                for ct in range(KT):
                    nc.tensor.matmul(
                        pv[:, 0:GF],
                        xt_sb[ct][:, tt * 128 : (tt + 1) * 128],
                        wv_sb[ct][:],
                        start=(ct == 0),
                        stop=(ct == KT - 1),
                    )
                v_t = p_v.tile([128, GH * (D + 1)], BF16, tag="v", name="v_t")
                vsrc = pv[:, 0:GF].rearrange("p (h d) -> p h d", h=GH)
                vdst = v_t[:].rearrange("p (h d) -> p h d", h=GH, d=D + 1)
                nc.vector.tensor_copy(vdst[:, :, 0:D], vsrc)
                nc.vector.memset(vdst[:, :, D : D + 1], 1.0)
                v_sb.append(v_t)

            vb_state = [0]

            def emit_proj_group(w_sb, eb_sb, dst_tiles, mt, n_v, post_mm=None,
                                jit_load=False):
                # weight-stationary proj: each weight tile streamed against
                # all ntb token blocks (4 live psum accumulators)
                pqs_j = [
                    p_oacc.tile([128, TB], F32, tag="oacc", name=f"pq{j}")
                    for j in range(ntb)
                ]
                for ct in range(KT):
                    if jit_load:
                        load_qx(ct + 1)
                    for j in range(ntb):
                        nc.tensor.matmul(
                            pqs_j[j][:],
                            w_sb[ct][:, mt * 128 : (mt + 1) * 128],
                            xt_sb[ct][:, j * TB : (j + 1) * TB],
                            start=(ct == 0),
                            stop=(ct == KT - 1),
                        )
                if post_mm is not None:
                    post_mm()
                # pass 1: all Squares + pss matmuls back-to-back on PE so the
                # skinny ACT/DVE stats pipeline behind them without PE gaps
                pss_j = []
                for j in range(ntb):
                    sq = p_tmp.tile([128, TB], BF16, tag=f"sq{j}", name=f"sq{j}")
                    nc.scalar.activation(sq[:], pqs_j[j][:], AF.Square)
                    pss = p_mm.tile([128, TB], F32, tag="mm", name=f"pss{j}")
                    nc.tensor.matmul(pss[0:2, :], blk2_sb[:], sq[:], start=True, stop=True)
                    pss_j.append(pss)
                # pass 2: chain tails (+ v-proj filler where available)
                for j in range(ntb):
                    emit_chain_tail(
                        pqs_j[j], pss_j[j], dst_tiles[mt],
                        slice(j * TB, (j + 1) * TB), eb_sb
                    )
                    for _ in range(2):
                        if vb_state[0] < min(ntt, n_v):
                            emit_v(vb_state[0])
                            vb_state[0] += 1

            # ---- attention -------------------------------------------------
            bounce = []
            for hp in range(2):
                bin_t = p_dram.tile([8 * 128, tsl], BF16, tag=f"bin{hp}")
                bout_t = p_dram.tile([8 * 128, tsl], BF16, tag=f"bout{hp}")
                bounce.append((bin_t, bout_t))

            def send_slices(hp, y_t, j):
                bin_t = bounce[hp][0]
                s0 = (j * TB) // tsl
                s1 = ((j + 1) * TB) // tsl
                for s in range(s0, s1):
                    nc.gpsimd.dma_start(
                        bin_t[s * 128 : (s + 1) * 128, :],
                        y_t[:, s * tsl : (s + 1) * tsl],
                    )

            def emit_attention(hp):
                # 1-query-block waves with double-buffered po accumulators
                # (p_oacc bufs=4 = 2 waves x 2 tiles) and a flat cross-wave
                # pipeline: attn_v lags LAG (tt) steps globally, so the next
                # wave's scores run while the previous wave drains.
                y_t = p_y.tile([128, t], BF16, tag="y")
                po_of = {}

                def emit_norm(j):
                    po = po_of[j]
                    rec = []
                    for hl in range(2):
                        dn = p_tmp.tile([1, TB], F32, tag=f"den{hl}", name=f"dn{hl}")
                        nc.vector.tensor_copy(dn[:], po[hl][64:65, :])
                        rf = p_tmp.tile([1, TB], F32, tag=f"recf{hl}", name=f"rf{hl}")
                        nc.vector.reciprocal_approx_fast(out=rf[:], in_=dn[:])
                        rc = p_tmp.tile([1, TB], BF16, tag=f"rec{hl}", name=f"rc{hl}")
                        nc.vector.tensor_copy(rc[:], rf[:])
                        rec.append(rc)
                    jb = slice(j * TB, (j + 1) * TB)
                    r_sb = p_tmp.tile([128, TB], BF16, tag="rsb", name="r_sb")
                    pr = p_mm.tile([128, TB], F32, tag="mm", name="pr")
                    nc.tensor.matmul(
                        pr[0:64, :], ones64[:], rec[0][:], start=True,
                        stop=True, tile_position=(0, 0),
                    )
                    nc.tensor.matmul(
                        pr[64:128, :], ones64[:], rec[1][:], start=True,
                        stop=True, tile_position=(0, 64),
                    )
                    nc.vector.tensor_copy(r_sb[:], pr[:])
                    nc.vector.tensor_mul(
                        y_t[0:64, jb], po[0][0:64, :], r_sb[0:64, :]
                    )
                    nc.vector.tensor_mul(
                        y_t[64:128, jb], po[1][0:64, :], r_sb[64:128, :]
                    )
                    send_slices(hp, y_t, j)

                def attn_v(j, tt, pts):
                    r = tt - 4 * j
                    lo = max(0, r) * 128
                    for hl in range(2):
                        h = 2 * hp + hl
                        nc.tensor.matmul(
                            po_of[j][hl][:, lo:TB],
                            v_sb[tt][:, h * (D + 1) : (h + 1) * (D + 1)],
                            pts[hl][:, lo:TB],
                            start=(tt == 0),
                            stop=(tt == 4 * (j + 1) - 1),
                            skip_group_check=True,
                        )
                    if tt == 4 * (j + 1) - 1:
                        emit_norm(j)

                LAG = 8
                pend = []
                for j in range(ntb):
                    po_of[j] = [
                        p_oacc.tile([D + 1, TB], F32, tag="oacc", name=f"po{j}_{i}")
                        for i in range(2)
                    ]
                    for tt in range(4 * (j + 1)):
                        r = tt - 4 * j
                        lo = max(0, r) * 128
                        pts = {}
                        for hl in range(2):
                            hofs = hl * 64
                            p = p_mm.tile([128, TB], F32, tag="mm", name=f"ps{hl}")
                            nc.tensor.matmul(
                                p[:, lo:TB],
                                kh_sb[hp][hofs : hofs + 64, tt * 128 : (tt + 1) * 128],
                                qh_sb[hp][hofs : hofs + 64, j * TB + lo : (j + 1) * TB],
                                start=True,
                                stop=True,
                                tile_position=(hofs, 0),
                            )
                            pt = p_pt.tile([128, TB], BF16, tag="pt")
                            nc.scalar.activation(
                                pt[:, lo:TB],
                                p[:, lo:TB],
                                AF.Exp,
                                scale=1.0 / np.sqrt(D),
                            )
                            if r >= 0:  # diagonal: triangle mask strip.
                                # gpsimd (mostly idle; tensor_tensor is in the
                                # default `standard` library now that no other
                                # ISA lib is in use)
                                nc.gpsimd.tensor_mul(
                                    pt[:, lo : lo + 128],
                                    pt[:, lo : lo + 128],
                                    mask_sb[:],
                                )
                            pts[hl] = pt
                        pend.append((j, tt, pts))
                        if len(pend) > LAG:
                            attn_v(*pend.pop(0))
                for item in pend:
                    attn_v(*item)

                # 8-way AllToAll: slot s carries our hp-features for token
                # window [s*tsl, (s+1)*tsl) of our batch; rank r receives,
                # from every rank s, (batch s//4, group s%4) features for
                # ITS window r -- all slots useful.
                bin_t, bout_t = bounce[hp]
                nc.gpsimd.collective_compute(
                    "AllToAll",
                    mybir.AluOpType.bypass,
                    ins=[bin_t.opt()],
                    outs=[bout_t.opt()],
                    replica_groups=[[0, 1, 2, 3, 4, 5, 6, 7]],
                )

            # ---- o_proj ----------------------------------------------------
            pw = max(2 * tsl, 512)  # pad pouts to a full psum bank
            pouts = []
            yg_sb = {}

            def alloc_pouts():
                for co in range(KT):
                    pool = p_oacc if co < 4 else p_mm
                    pouts.append(pool.tile(
                        [128, pw], F32, tag="oacc" if co < 4 else "mm",
                        name=f"pout{co}"))

            def emit_oproj_half(hp):
                # one readback DMA per hp: bout [8*128, tsl] -> [128, 8*tsl]
                bout_t = bounce[hp][1]
                yg_t = p_yg.tile([128, 8 * tsl], BF16, tag="yg", name=f"yg{hp}")
                src_ap = bout_t[:].rearrange("(s p) c -> p s c", s=8)
                dst_ap = yg_t[:].rearrange("p (s c) -> p s c", s=8)
                nc.sync.dma_start(dst_ap, src_ap)
                yg_sb[hp] = yg_t
                # out^T[cout, 2*tsl] (cols [0,tsl)=batch0).  "start" pending-
                # zeroes the whole 2KB psum bank: only the first matmul per
                # tile sets it; the b=1 region is lazily zeroed on first
                # touch.
                for co in range(KT):
                    for b in range(2):
                        for g in range(GH):
                            m = 2 * g + hp
                            s = 4 * b + g
                            nc.tensor.matmul(
                                pouts[co][:, b * tsl : (b + 1) * tsl],
                                wo_sb[m][:, co * 128 : (co + 1) * 128],
                                yg_sb[hp][:, s * tsl : (s + 1) * tsl],
                                start=(hp == 0 and b == 0 and g == 0),
                                stop=(hp == 1 and g == GH - 1),
                                skip_group_check=True,
                            )
                    if hp == 1:
                        o_sb = p_osb.tile([128, 2 * tsl], BF16, tag="osb")
                        if co % 2 == 0:
                            nc.vector.tensor_copy(o_sb[:], pouts[co][:, 0 : 2 * tsl])
                        else:
                            nc.scalar.copy(o_sb[:], pouts[co][:, 0 : 2 * tsl])
                        eng = nc.sync if co % 2 == 0 else nc.scalar
                        eng.dma_start(out[co * 128 : (co + 1) * 128, :], o_sb[:])

            # ---- main schedule --------------------------------------------
            # interleave mt0 projections / attention hp0 / mt1 projections /
            # attention hp1 so PE never sits behind straggler chains, with
            # loads emitted just-in-time
            def post_g1():
                load_tables()
                load_wv()
                load_mask()

            emit_proj_group(wq_sb, eqb_sb, qh_sb, 0, ntt // 2, post_mm=post_g1,
                            jit_load=True)
            load_wk()
            emit_proj_group(wk_sb, ekb_sb, kh_sb, 0, ntt)
            emit_attention(0)
            load_wo()
            emit_proj_group(wq_sb, eqb_sb, qh_sb, 1, ntt)
            emit_proj_group(wk_sb, ekb_sb, kh_sb, 1, ntt)
            emit_attention(1)
            alloc_pouts()
            emit_oproj_half(0)
            emit_oproj_half(1)

    nc.compile()
    return nc


# ---------------------------------------------------------------------------
# host side
# ---------------------------------------------------------------------------

# rope permutation: new position p (within a head's 64 rows) -> original d.
# layout per 64: [xr0..15, xi0..15, xr16..31, xi16..31] so rope partners are
# +/-16 within each 32-block (stream_shuffle-able).
def _perm_d():
    d_of_p = np.empty(64, np.int64)
    for p in range(64):
        f = 16 * (p // 32) + (p % 16)
        d_of_p[p] = f if (p % 32) < 16 else 32 + f
    return d_of_p


PERM_D = _perm_d()


def _rope_tables(t):
    inv_freq = 1.0 / (ROPE_BASE ** (np.arange(0, D, 2, dtype=np.float64) / D))  # [32]
    ang = np.arange(t, dtype=np.float64)[:, None] * inv_freq[None, :]  # [t, 32]
    cos = np.cos(ang).astype(np.float32)
    sin = np.sin(ang).astype(np.float32)
    cosf = np.empty((128, t), np.float32)
    sinf = np.empty((128, t), np.float32)
    for r in range(128):
        p = r % 64
        f = 16 * (p // 32) + (p % 16)
        is_xi = (p % 32) >= 16
        cosf[r] = cos[:, f]
        sinf[r] = sin[:, f] if is_xi else -sin[:, f]
    return cosf, sinf


def _consts(t):
    cosf, sinf = _rope_tables(t)
    blk2 = np.zeros((128, 2), np.float32)
    blk2[0:64, 0] = 1.0
    blk2[64:128, 1] = 1.0
    maskt = np.zeros((128, 128), np.float32)
    for p in range(128):
        maskt[p, p:] = 1.0
    return cosf, sinf, blk2, maskt


def _pswap16():
    ps = np.zeros((128, 128), np.float32)
    for j in range(128):
        i = (j // 32) * 32 + ((j % 32) + 16) % 32
        ps[i, j] = 1.0
    return ps


def _eb(w):
    wp = np.asarray(w, np.float32)[PERM_D]
    e = np.zeros((2, 128), np.float32)
    e[0, 0:64] = wp
    e[1, 64:128] = wp
    return e


def _wcol(w):
    # norm weight as per-partition column, rope-permuted, tiled for 2 heads
    wp = np.asarray(w, np.float32)[PERM_D]
    return np.concatenate([wp, wp]).reshape(128, 1)


def _bf(x):
    return np.ascontiguousarray(x).astype(ml_dtypes.bfloat16)


def _perm_wqk(wt):
    # wt: [C_in, GF] (transposed weight slice); permute output features so
    # each head's 64 columns follow the rope-shuffle layout
    wt = wt.reshape(wt.shape[0], GH, D)
    return np.ascontiguousarray(wt[:, :, PERM_D].reshape(wt.shape[0], GH * D))


def make_in_maps(x, Wq, Wk, Wv, Wo, qn_w, kn_w, t=T):
    cosf, sinf, blk2, maskt = _consts(t)
    common = {
        "cosf": _bf(cosf),
        "sinf": _bf(sinf),
        "blk2": _bf(blk2),
        "maskt": _bf(maskt),
        "pswap": _bf(_pswap16()),
        "eqb": _bf(_eb(qn_w)),
        "ekb": _bf(_eb(kn_w)),
        "wo": _bf(Wo.T),  # [c_in, c_out]
    }
    in_maps = []
    for c in range(N_CORES):
        b, g = c // 4, c % 4
        fs = slice(GF * g, GF * (g + 1))
        in_maps.append(
            dict(
                common,
                xt=_bf(x[b, :t, :].T),
                wq=_bf(_perm_wqk(Wq[fs, :].T)),
                wk=_bf(_perm_wqk(Wk[fs, :].T)),
                wv=_bf(Wv[fs, :].T),
            )
        )
    return in_maps


def assemble(results, t=T):
    tsl = t // 8
    out = np.empty((B, t, C), np.float32)
    for c in range(N_CORES):
        res = results[c]["out"]  # [C, 2*tsl]
        out[0, c * tsl : (c + 1) * tsl, :] = res[:, 0:tsl].T
        out[1, c * tsl : (c + 1) * tsl, :] = res[:, tsl : 2 * tsl].T
    return out


# -- cached PJRT runner (compile once, reuse across kernel() calls) ---------

_RUNNER = {}


def _get_runner(t=T):
    if t in _RUNNER:
        return _RUNNER[t]
    import jax
    from jax.sharding import Mesh, PartitionSpec
    from jax.experimental.shard_map import shard_map
    from concourse import bass2jax

    nc = build_nc(t)
    bass2jax.install_neuronx_cc_hook()

    partition_name = nc.partition_id_tensor.name if nc.partition_id_tensor else None
    in_names = []
    out_names = []
    out_avals = []
    zero_outs = []
    for alloc in nc.m.functions[0].allocations:
        if not isinstance(alloc, mybir.MemoryLocationSet):
            continue
        name = alloc.memorylocations[0].name
        if alloc.kind == "ExternalInput":
            if name == partition_name:
                continue
            in_names.append(name)
        elif alloc.kind == "ExternalOutput":
            shape = tuple(alloc.tensor_shape)
            dtype = mybir.dt.np(alloc.dtype)
            out_names.append(name)
            out_avals.append(jax.core.ShapedArray(shape, dtype))
            zero_outs.append(np.zeros(shape, dtype))
    n_params = len(in_names)
    all_names = in_names + out_names
    if partition_name is not None:
        all_names = all_names + [partition_name]

    def _body(*args):
        operands = list(args)
        if partition_name is not None:
            operands.append(bass2jax.partition_id_tensor())
        outs = bass2jax._bass_exec_p.bind(
            *operands,
            out_avals=tuple(out_avals),
            in_names=tuple(all_names),
            out_names=tuple(out_names),
            lowering_input_output_aliases=(),
            sim_require_finite=True,
            sim_require_nnan=True,
            nc=nc,
        )
        return tuple(outs)

    devices = jax.devices()[:N_CORES]
    mesh = Mesh(np.asarray(devices), ("core",))
    fn = jax.jit(
        shard_map(
            _body,
            mesh=mesh,
            in_specs=(PartitionSpec("core"),) * (n_params + len(out_names)),
            out_specs=(PartitionSpec("core"),) * len(out_names),
            check_rep=False,
        ),
        keep_unused=True,
    )
    runner = {
        "fn": fn,
        "body": _body,
        "in_names": in_names,
        "out_names": out_names,
        "out_avals": out_avals,
        "zero_outs": zero_outs,
        "jax": jax,
    }
    _RUNNER[t] = runner
    return runner


def run_device(in_maps, t=T):
    r = _get_runner(t)
    concat_in = [
        np.concatenate([np.asarray(m[name]) for m in in_maps], axis=0)
        for name in r["in_names"]
    ]
    concat_zero = [
        np.zeros((N_CORES * z.shape[0], *z.shape[1:]), z.dtype) for z in r["zero_outs"]
    ]
    outs = r["fn"](*concat_in, *concat_zero)
    results = []
    for c in range(N_CORES):
        results.append(
            {
                name: np.asarray(outs[i]).reshape(N_CORES, *r["out_avals"][i].shape)[c]
                for i, name in enumerate(r["out_names"])
            }
        )
    return results


def kernel(x, Wq, Wk, Wv, Wo, qn_w, kn_w):
    x = np.asarray(x, np.float32)
    in_maps = make_in_maps(
        x,
        np.asarray(Wq, np.float32),
        np.asarray(Wk, np.float32),
        np.asarray(Wv, np.float32),
        np.asarray(Wo, np.float32),
        np.asarray(qn_w, np.float32),
        np.asarray(kn_w, np.float32),
    )
    results = run_device(in_maps)
    return assemble(results)


# revision 32
# speedup vs baseline: 1.0319x; 1.0319x over previous
"""Trainium2 Bass kernel for causal self-attention (RoPE + per-head RMSNorm).

Reference computation (B=2, T=2048, C=1024, H=16, D=64):
    q = rope(rmsnorm(x @ Wq.T)); k = rope(rmsnorm(x @ Wk.T)); v = x @ Wv.T
    out = softmax(causal(q k^T / sqrt(D))) v @ Wo.T

Sharding over 8 NeuronCores: core c -> batch b = c//4, head-group g = c%4
(4 heads = 256 features per group).  Everything on-chip is computed in a
feature-major ("transposed") layout so no PE transposes are needed:
  - scores are computed as S^T[tk, tq] tiles, softmax runs over the
    partition axis using matmul-with-ones tricks (denominator comes from a
    ones column appended to V), and the final division is applied via a
    gpsimd partition_broadcast + DVE multiply.
  - causal structure is exploited at the instruction level: score /
    exp / attn_v instructions on diagonal tiles only cover the valid
    column window; the triangle mask is a single [128,128] multiply.
  - rope's half-rotation is a DVE stream_shuffle: the host permutes the
    W{q,k} output features so rope partners live in the same 32-partition
    quadrant (scores are invariant to a consistent permutation of d).
  - attention output Y^T (feature-major) is exchanged with an AllToAll
    over all 8 cores using token-slices of T/8 per slot, so every slot
    carries useful data; rank r then runs o_proj for token window
    [r*T/8, (r+1)*T/8) of BOTH batches.  The o_proj is split into an
    hp0 half (runs while the second AllToAll is in flight) and an hp1
    half, accumulating in PSUM across the two.
Host side: shards/permutes/transposes inputs (bf16), assembles fp32 output.
"""

import os
import sys

for _p in ("/opt/trn_rl_repo", "/root/.axon_site/_ro/trn_rl_repo"):
    if os.path.isdir(_p) and _p not in sys.path:
        sys.path.insert(0, _p)

import numpy as np
import ml_dtypes

import concourse.bass as bass
from concourse import bacc
import concourse.tile as tile
import concourse.mybir as mybir

BF16 = mybir.dt.bfloat16
F32 = mybir.dt.float32
AF = mybir.ActivationFunctionType
ALU = mybir.AluOpType

B, T, C, H, D = 2, 2048, 1024, 16, 64
N_CORES = 8
GH = 4  # heads per core
GF = GH * D  # features per core (256)
TB = 512  # token block (matmul N)
KT = C // 128  # 8 contraction k-tiles
EPS = float(np.finfo(np.float32).eps)
ROPE_BASE = 10000.0

# stream_shuffle mask: swap 16-row halves within each 32-partition quadrant
SHUF16 = list(range(16, 32)) + list(range(0, 16))

# bisection switches
NO_PB = bool(int(os.environ.get("V2_NO_PB", "1")))        # matmul broadcasts
SYNC_LOADS = bool(int(os.environ.get("V2_SYNC_LOADS", "0")))  # loads on SP
NO_SHUF = bool(int(os.environ.get("V2_NO_SHUF", "0")))    # pswap matmul rope
DEBUG_DUMP = bool(int(os.environ.get("V2_DEBUG", "0")))   # dump q/k/y intermediates


def build_nc(t=T):
    ntb = t // TB  # tq blocks
    ntt = t // 128  # token 128-tiles
    tsl = t // 8  # per-rank token slice for o_proj (per batch)

    nc = bacc.Bacc("TRN2", target_bir_lowering=False, debug=False, num_devices=N_CORES)

    xt = nc.dram_tensor("xt", [C, t], BF16, kind="ExternalInput")
    wq = nc.dram_tensor("wq", [C, GF], BF16, kind="ExternalInput")
    wk = nc.dram_tensor("wk", [C, GF], BF16, kind="ExternalInput")
    wv = nc.dram_tensor("wv", [C, GF], BF16, kind="ExternalInput")
    wo = nc.dram_tensor("wo", [C, C], BF16, kind="ExternalInput")
    cosf = nc.dram_tensor("cosf", [128, t], BF16, kind="ExternalInput")
    sinf = nc.dram_tensor("sinf", [128, t], BF16, kind="ExternalInput")
    blk2 = nc.dram_tensor("blk2", [128, 2], BF16, kind="ExternalInput")
    maskt = nc.dram_tensor("maskt", [128, 128], BF16, kind="ExternalInput")
    pswap = nc.dram_tensor("pswap", [128, 128], BF16, kind="ExternalInput")
    eqb = nc.dram_tensor("eqb", [2, 128], BF16, kind="ExternalInput")
    ekb = nc.dram_tensor("ekb", [2, 128], BF16, kind="ExternalInput")
    out = nc.dram_tensor("out", [C, 2 * tsl], BF16, kind="ExternalOutput")
    if DEBUG_DUMP:
        qdbg = nc.dram_tensor("qdbg", [2 * 128, t], F32, kind="ExternalOutput")
        kdbg = nc.dram_tensor("kdbg", [2 * 128, t], F32, kind="ExternalOutput")
        ydbg = nc.dram_tensor("ydbg", [2 * 128, t], F32, kind="ExternalOutput")
        ygdbg = nc.dram_tensor("ygdbg", [16 * 128, tsl], F32, kind="ExternalOutput")

    with tile.TileContext(nc) as tc:
        with (
            nc.allow_low_precision(reason="bf16 compute by design"),
            tc.tile_pool(name="p_xt", bufs=KT) as p_xt,
            tc.tile_pool(name="p_w", bufs=KT) as p_w,
            tc.tile_pool(name="p_wo", bufs=KT) as p_wo,
            tc.tile_pool(name="p_tab", bufs=1) as p_tab,
            tc.tile_pool(name="p_qk", bufs=2) as p_qk,
            tc.tile_pool(name="p_v", bufs=ntt) as p_v,
            tc.tile_pool(name="p_y", bufs=2) as p_y,
            tc.tile_pool(name="p_yg", bufs=2) as p_yg,
            tc.tile_pool(name="p_pt", bufs=22) as p_pt,
            tc.tile_pool(name="p_tmp", bufs=2) as p_tmp,
            tc.tile_pool(name="p_osb", bufs=4) as p_osb,
            tc.tile_pool(name="p_mm", bufs=4, space="PSUM") as p_mm,
            tc.tile_pool(name="p_oacc", bufs=4, space="PSUM") as p_oacc,
            tc.tile_pool(name="p_dram", bufs=2, space="DRAM") as p_dram,
        ):
            # ---- load inputs -------------------------------------------------
            # Deps are aggregated counting-semaphore thresholds keyed on
            # emission order, so loads are emitted just-in-time: anything
            # emitted before a PE instruction serializes ahead of it.
            xt_sb = [None] * KT
            wq_sb = [None] * KT
            wk_sb = [None] * KT
            wv_sb = []
            x_t = p_xt.tile([128, t], BF16, tag="xt", name="xt0")
            nc.sync.dma_start(x_t[:], xt[0:128, :])
            xt_sb[0] = x_t
            w_t = p_w.tile([128, GF], BF16, tag="wq")
            nc.sync.dma_start(w_t[:], wq[0:128, :])
            wq_sb[0] = w_t

            def load_qx(ct):
                # just-in-time load of (wq[ct], xt[ct]) pairs
                if ct >= KT or xt_sb[ct] is not None:
                    return
                eng = nc.sync if ct % 2 == 0 else nc.scalar
                w_t = p_w.tile([128, GF], BF16, tag="wq")
                eng.dma_start(w_t[:], wq[ct * 128 : (ct + 1) * 128, :])
                wq_sb[ct] = w_t
                x_t = p_xt.tile([128, t], BF16, tag="xt", name=f"xt{ct}")
                eng.dma_start(x_t[:], xt[ct * 128 : (ct + 1) * 128, :])
                xt_sb[ct] = x_t

            cos_sb = p_tab.tile([128, t], BF16, tag="cos")
            sin_sb = p_tab.tile([128, t], BF16, tag="sin")
            blk2_sb = p_tab.tile([128, 2], BF16, tag="blk2")
            eqb_sb = p_tab.tile([2, 128], BF16, tag="eqb")
            ekb_sb = p_tab.tile([2, 128], BF16, tag="ekb")

            def load_tables():
                nc.scalar.dma_start(cos_sb[:], cosf[:])
                nc.scalar.dma_start(sin_sb[:], sinf[:])
                nc.scalar.dma_start(blk2_sb[:], blk2[:])
                nc.scalar.dma_start(eqb_sb[:], eqb[:])
                nc.scalar.dma_start(ekb_sb[:], ekb[:])
                if NO_SHUF:
                    nc.scalar.dma_start(pswap_sb[:], pswap[:])

            if NO_SHUF:
                pswap_sb = p_tab.tile([128, 128], BF16, tag="pswap")
            if NO_PB:
                ones64 = p_tab.tile([1, 64], BF16, tag="ones64")
                nc.vector.memset(ones64[:], 1.0)
            eps_sb = p_tab.tile([2, 1], F32, tag="eps")
            nc.vector.memset(eps_sb[:], EPS)

            def load_wv():
                for ct in range(KT):
                    w_t = p_w.tile([128, GF], BF16, tag="wv")
                    eng = nc.sync if ct % 2 == 0 else nc.scalar
                    eng.dma_start(w_t[:], wv[ct * 128 : (ct + 1) * 128, :])
                    wv_sb.append(w_t)

            def load_wk():
                for ct in range(KT):
                    w_t = p_w.tile([128, GF], BF16, tag="wk")
                    eng = nc.sync if ct % 2 == 0 else nc.scalar
                    eng.dma_start(w_t[:], wk[ct * 128 : (ct + 1) * 128, :])
                    wk_sb[ct] = w_t

            mask_sb = None
            wo_sb = []

            def load_mask():
                nonlocal mask_sb
                mask_sb = p_tab.tile([128, 128], BF16, tag="mask")
                nc.scalar.dma_start(mask_sb[:], maskt[:])

            def load_wo():
                for mt in range(KT):
                    w_t = p_wo.tile([128, C], BF16, tag="wo")
                    eng = nc.sync if mt % 2 == 0 else nc.scalar
                    eng.dma_start(w_t[:], wo[mt * 128 : (mt + 1) * 128, :])
                    wo_sb.append(w_t)

            # ---- q/k/v projections, rmsnorm + rope (software-pipelined) -----
            qh_sb = [p_qk.tile([128, t], BF16, tag="qk0", name="qh0"),
                     p_qk.tile([128, t], BF16, tag="qk1", name="qh1")]
            kh_sb = [p_qk.tile([128, t], BF16, tag="qk0", name="kh0"),
                     p_qk.tile([128, t], BF16, tag="qk1", name="kh1")]
            v_sb = []

            def emit_chain_tail(pq, pss, dst, jb, eb_sb):
                # skinny rms stats -> broadcast matmul -> rope (Square and
                # the pss matmul were emitted earlier, see emit_proj_group)
                sqm = p_tmp.tile([2, TB], F32, tag="sqm", name="sqm")
                nc.scalar.activation(
                    sqm[:], pss[0:2, :], AF.Sqrt, scale=1.0 / D, bias=eps_sb[:]
                )
                invf = p_tmp.tile([2, TB], F32, tag="invf", name="invf")
                nc.vector.reciprocal_approx_fast(out=invf[:], in_=sqm[:])
                inv = p_tmp.tile([2, TB], BF16, tag="inv", name="inv")
                nc.vector.tensor_copy(inv[:], invf[:])
                pinvb = p_mm.tile([128, TB], F32, tag="mm", name="pinvb")
                nc.tensor.matmul(pinvb[:], eb_sb[:], inv[:], start=True, stop=True)
                invb = p_tmp.tile([128, TB], BF16, tag="invb", name="invb")
                nc.vector.tensor_copy(invb[:], pinvb[:])
                qn = p_tmp.tile([128, TB], BF16, tag="qn", name="qn")
                nc.vector.tensor_mul(qn[:], pq[:], invb[:])
                # rope: dst = qn*cos + swap(qn)*sin_signed
                qsw = p_tmp.tile([128, TB], BF16, tag="qsw", name="qsw")
                if NO_SHUF:
                    pqs = p_mm.tile([128, TB], F32, tag="mm", name="pqs")
                    nc.tensor.matmul(pqs[:], pswap_sb[:], qn[:], start=True, stop=True)
                    nc.vector.tensor_copy(qsw[:], pqs[:])
                else:
                    nc.vector.stream_shuffle(qsw[:], qn[:], SHUF16)
                t1 = p_tmp.tile([128, TB], BF16, tag="t1", name="t1")
                nc.vector.tensor_mul(t1[:], qn[:], cos_sb[:, jb])
                t2 = p_tmp.tile([128, TB], BF16, tag="t2", name="t2")
                nc.vector.tensor_mul(t2[:], qsw[:], sin_sb[:, jb])
                nc.vector.tensor_add(dst[:, jb], t1[:], t2[:])

            def emit_v(tt):
                pv = p_mm.tile([128, TB], F32, tag="mm", name="pv")
                for ct in range(KT):
                    nc.tensor.matmul(
                        pv[:, 0:GF],
                        xt_sb[ct][:, tt * 128 : (tt + 1) * 128],
                        wv_sb[ct][:],
                        start=(ct == 0),
                        stop=(ct == KT - 1),
                    )
                v_t = p_v.tile([128, GH * (D + 1)], BF16, tag="v", name="v_t")
                vsrc = pv[:, 0:GF].rearrange("p (h d) -> p h d", h=GH)
                vdst = v_t[:].rearrange("p (h d) -> p h d", h=GH, d=D + 1)
                nc.vector.tensor_copy(vdst[:, :, 0:D], vsrc)
                nc.vector.memset(vdst[:, :, D : D + 1], 1.0)
                v_sb.append(v_t)

            vb_state = [0]

            def emit_proj_group(w_sb, eb_sb, dst_tiles, mt, n_v, post_mm=None,
                                jit_load=False):
                # weight-stationary proj: each weight tile streamed against
                # all ntb token blocks (4 live psum accumulators)
                pqs_j = [
                    p_oacc.tile([128, TB], F32, tag="oacc", name=f"pq{j}")
                    for j in range(ntb)
                ]
                for ct in range(KT):
                    if jit_load:
                        load_qx(ct + 1)
                    for j in range(ntb):
                        nc.tensor.matmul(
                            pqs_j[j][:],
                            w_sb[ct][:, mt * 128 : (mt + 1) * 128],
                            xt_sb[ct][:, j * TB : (j + 1) * TB],
                            start=(ct == 0),
                            stop=(ct == KT - 1),
                        )
                if post_mm is not None:
                    post_mm()
                # pass 1: all Squares + pss matmuls back-to-back on PE so the
                # skinny ACT/DVE stats pipeline behind them without PE gaps
                pss_j = []
                for j in range(ntb):
                    sq = p_tmp.tile([128, TB], BF16, tag=f"sq{j}", name=f"sq{j}")
                    nc.scalar.activation(sq[:], pqs_j[j][:], AF.Square)
                    pss = p_mm.tile([128, TB], F32, tag="mm", name=f"pss{j}")
                    nc.tensor.matmul(pss[0:2, :], blk2_sb[:], sq[:], start=True, stop=True)
                    pss_j.append(pss)
                # pass 2: chain tails (+ v-proj filler where available)
                for j in range(ntb):
                    emit_chain_tail(
                        pqs_j[j], pss_j[j], dst_tiles[mt],
                        slice(j * TB, (j + 1) * TB), eb_sb
                    )
                    for _ in range(2):
                        if vb_state[0] < min(ntt, n_v):
                            emit_v(vb_state[0])
                            vb_state[0] += 1

            # ---- attention -------------------------------------------------
            bounce = []
            for hp in range(2):
                bin_t = p_dram.tile([8 * 128, tsl], BF16, tag=f"bin{hp}")
                bout_t = p_dram.tile([8 * 128, tsl], BF16, tag=f"bout{hp}")
                bounce.append((bin_t, bout_t))

            def send_slices(hp, y_t, j):
                bin_t = bounce[hp][0]
                s0 = (j * TB) // tsl
                s1 = ((j + 1) * TB) // tsl
                for s in range(s0, s1):
                    nc.gpsimd.dma_start(
                        bin_t[s * 128 : (s + 1) * 128, :],
                        y_t[:, s * tsl : (s + 1) * tsl],
                    )

            def emit_attention(hp):
                # 1-query-block waves with double-buffered po accumulators
                # (p_oacc bufs=4 = 2 waves x 2 tiles) and a flat cross-wave
                # pipeline: attn_v lags LAG (tt) steps globally, so the next
                # wave's scores run while the previous wave drains.
                y_t = p_y.tile([128, t], BF16, tag="y")
                po_of = {}

                def emit_norm(j):
                    po = po_of[j]
                    rec = []
                    for hl in range(2):
                        dn = p_tmp.tile([1, TB], F32, tag=f"den{hl}", name=f"dn{hl}")
                        nc.vector.tensor_copy(dn[:], po[hl][64:65, :])
                        rf = p_tmp.tile([1, TB], F32, tag=f"recf{hl}", name=f"rf{hl}")
                        nc.vector.reciprocal_approx_fast(out=rf[:], in_=dn[:])
                        rc = p_tmp.tile([1, TB], BF16, tag=f"rec{hl}", name=f"rc{hl}")
                        nc.vector.tensor_copy(rc[:], rf[:])
                        rec.append(rc)
                    jb = slice(j * TB, (j + 1) * TB)
                    r_sb = p_tmp.tile([128, TB], BF16, tag="rsb", name="r_sb")
                    pr = p_mm.tile([128, TB], F32, tag="mm", name="pr")
                    nc.tensor.matmul(
                        pr[0:64, :], ones64[:], rec[0][:], start=True,
                        stop=True, tile_position=(0, 0),
                    )
                    nc.tensor.matmul(
                        pr[64:128, :], ones64[:], rec[1][:], start=True,
                        stop=True, tile_position=(0, 64),
                    )
                    nc.vector.tensor_copy(r_sb[:], pr[:])
                    nc.vector.tensor_mul(
                        y_t[0:64, jb], po[0][0:64, :], r_sb[0:64, :]
                    )
                    nc.vector.tensor_mul(
                        y_t[64:128, jb], po[1][0:64, :], r_sb[64:128, :]
                    )
                    send_slices(hp, y_t, j)

                def attn_v(j, tt, pts):
                    r = tt - 4 * j
                    lo = max(0, r) * 128
                    for hl in range(2):
                        h = 2 * hp + hl
                        nc.tensor.matmul(
                            po_of[j][hl][:, lo:TB],
                            v_sb[tt][:, h * (D + 1) : (h + 1) * (D + 1)],
                            pts[hl][:, lo:TB],
                            start=(tt == 0),
                            stop=(tt == 4 * (j + 1) - 1),
                            skip_group_check=True,
                        )
                    if tt == 4 * (j + 1) - 1:
                        emit_norm(j)

                LAG = 9
                pend = []
                for j in range(ntb):
                    po_of[j] = [
                        p_oacc.tile([D + 1, TB], F32, tag="oacc", name=f"po{j}_{i}")
                        for i in range(2)
                    ]
                    for tt in range(4 * (j + 1)):
                        r = tt - 4 * j
                        lo = max(0, r) * 128
                        pts = {}
                        for hl in range(2):
                            hofs = hl * 64
                            p = p_mm.tile([128, TB], F32, tag="mm", name=f"ps{hl}")
                            nc.tensor.matmul(
                                p[:, lo:TB],
                                kh_sb[hp][hofs : hofs + 64, tt * 128 : (tt + 1) * 128],
                                qh_sb[hp][hofs : hofs + 64, j * TB + lo : (j + 1) * TB],
                                start=True,
                                stop=True,
                                tile_position=(hofs, 0),
                            )
                            pt = p_pt.tile([128, TB], BF16, tag="pt")
                            nc.scalar.activation(
                                pt[:, lo:TB],
                                p[:, lo:TB],
                                AF.Exp,
                                scale=1.0 / np.sqrt(D),
                            )
                            if r >= 0:  # diagonal: triangle mask strip.
                                # gpsimd (mostly idle; tensor_tensor is in the
                                # default `standard` library now that no other
                                # ISA lib is in use)
                                nc.gpsimd.tensor_mul(
                                    pt[:, lo : lo + 128],
                                    pt[:, lo : lo + 128],
                                    mask_sb[:],
                                )
                            pts[hl] = pt
                        pend.append((j, tt, pts))
                        if len(pend) > LAG:
                            attn_v(*pend.pop(0))
                for item in pend:
                    attn_v(*item)

                # 8-way AllToAll: slot s carries our hp-features for token
                # window [s*tsl, (s+1)*tsl) of our batch; rank r receives,
                # from every rank s, (batch s//4, group s%4) features for
                # ITS window r -- all slots useful.
                bin_t, bout_t = bounce[hp]
                nc.gpsimd.collective_compute(
                    "AllToAll",
                    mybir.AluOpType.bypass,
                    ins=[bin_t.opt()],
                    outs=[bout_t.opt()],
                    replica_groups=[[0, 1, 2, 3, 4, 5, 6, 7]],
                )

            # ---- o_proj ----------------------------------------------------
            pw = max(2 * tsl, 512)  # pad pouts to a full psum bank
            pouts = []
            yg_sb = {}

            def alloc_pouts():
                for co in range(KT):
                    pool = p_oacc if co < 4 else p_mm
                    pouts.append(pool.tile(
                        [128, pw], F32, tag="oacc" if co < 4 else "mm",
                        name=f"pout{co}"))

            def emit_oproj_half(hp):
                # one readback DMA per hp: bout [8*128, tsl] -> [128, 8*tsl]
                bout_t = bounce[hp][1]
                yg_t = p_yg.tile([128, 8 * tsl], BF16, tag="yg", name=f"yg{hp}")
                src_ap = bout_t[:].rearrange("(s p) c -> p s c", s=8)
                dst_ap = yg_t[:].rearrange("p (s c) -> p s c", s=8)
                nc.sync.dma_start(dst_ap, src_ap)
                yg_sb[hp] = yg_t
                # out^T[cout, 2*tsl] (cols [0,tsl)=batch0).  "start" pending-
                # zeroes the whole 2KB psum bank: only the first matmul per
                # tile sets it; the b=1 region is lazily zeroed on first
                # touch.
                for co in range(KT):
                    for b in range(2):
                        for g in range(GH):
                            m = 2 * g + hp
                            s = 4 * b + g
                            nc.tensor.matmul(
                                pouts[co][:, b * tsl : (b + 1) * tsl],
                                wo_sb[m][:, co * 128 : (co + 1) * 128],
                                yg_sb[hp][:, s * tsl : (s + 1) * tsl],
                                start=(hp == 0 and b == 0 and g == 0),
                                stop=(hp == 1 and g == GH - 1),
                                skip_group_check=True,
                            )
                    if hp == 1:
                        o_sb = p_osb.tile([128, 2 * tsl], BF16, tag="osb")
                        if co % 2 == 0:
                            nc.vector.tensor_copy(o_sb[:], pouts[co][:, 0 : 2 * tsl])
                        else:
                            nc.scalar.copy(o_sb[:], pouts[co][:, 0 : 2 * tsl])
                        eng = nc.sync if co % 2 == 0 else nc.scalar
                        eng.dma_start(out[co * 128 : (co + 1) * 128, :], o_sb[:])

            # ---- main schedule --------------------------------------------
            # interleave mt0 projections / attention hp0 / mt1 projections /
            # attention hp1 so PE never sits behind straggler chains, with
            # loads emitted just-in-time
            def post_g1():
                load_tables()
                load_wv()
                load_mask()

            emit_proj_group(wq_sb, eqb_sb, qh_sb, 0, ntt // 2, post_mm=post_g1,
                            jit_load=True)
            load_wk()
            emit_proj_group(wk_sb, ekb_sb, kh_sb, 0, ntt)
            emit_attention(0)
            load_wo()
            emit_proj_group(wq_sb, eqb_sb, qh_sb, 1, ntt)
            emit_proj_group(wk_sb, ekb_sb, kh_sb, 1, ntt)
            emit_attention(1)
            alloc_pouts()
            emit_oproj_half(0)
            emit_oproj_half(1)

    nc.compile()
    return nc


# ---------------------------------------------------------------------------
# host side
# ---------------------------------------------------------------------------

# rope permutation: new position p (within a head's 64 rows) -> original d.
# layout per 64: [xr0..15, xi0..15, xr16..31, xi16..31] so rope partners are
# +/-16 within each 32-block (stream_shuffle-able).
def _perm_d():
    d_of_p = np.empty(64, np.int64)
    for p in range(64):
        f = 16 * (p // 32) + (p % 16)
        d_of_p[p] = f if (p % 32) < 16 else 32 + f
    return d_of_p


PERM_D = _perm_d()


def _rope_tables(t):
    inv_freq = 1.0 / (ROPE_BASE ** (np.arange(0, D, 2, dtype=np.float64) / D))  # [32]
    ang = np.arange(t, dtype=np.float64)[:, None] * inv_freq[None, :]  # [t, 32]
    cos = np.cos(ang).astype(np.float32)
    sin = np.sin(ang).astype(np.float32)
    cosf = np.empty((128, t), np.float32)
    sinf = np.empty((128, t), np.float32)
    for r in range(128):
        p = r % 64
        f = 16 * (p // 32) + (p % 16)
        is_xi = (p % 32) >= 16
        cosf[r] = cos[:, f]
        sinf[r] = sin[:, f] if is_xi else -sin[:, f]
    return cosf, sinf


def _consts(t):
    cosf, sinf = _rope_tables(t)
    blk2 = np.zeros((128, 2), np.float32)
    blk2[0:64, 0] = 1.0
    blk2[64:128, 1] = 1.0
    maskt = np.zeros((128, 128), np.float32)
    for p in range(128):
        maskt[p, p:] = 1.0
    return cosf, sinf, blk2, maskt


def _pswap16():
    ps = np.zeros((128, 128), np.float32)
    for j in range(128):
        i = (j // 32) * 32 + ((j % 32) + 16) % 32
        ps[i, j] = 1.0
    return ps


def _eb(w):
    wp = np.asarray(w, np.float32)[PERM_D]
    e = np.zeros((2, 128), np.float32)
    e[0, 0:64] = wp
    e[1, 64:128] = wp
    return e


def _wcol(w):
    # norm weight as per-partition column, rope-permuted, tiled for 2 heads
    wp = np.asarray(w, np.float32)[PERM_D]
    return np.concatenate([wp, wp]).reshape(128, 1)


def _bf(x):
    return np.ascontiguousarray(x).astype(ml_dtypes.bfloat16)


def _perm_wqk(wt):
    # wt: [C_in, GF] (transposed weight slice); permute output features so
    # each head's 64 columns follow the rope-shuffle layout
    wt = wt.reshape(wt.shape[0], GH, D)
    return np.ascontiguousarray(wt[:, :, PERM_D].reshape(wt.shape[0], GH * D))


def make_in_maps(x, Wq, Wk, Wv, Wo, qn_w, kn_w, t=T):
    cosf, sinf, blk2, maskt = _consts(t)
    common = {
        "cosf": _bf(cosf),
        "sinf": _bf(sinf),
        "blk2": _bf(blk2),
        "maskt": _bf(maskt),
        "pswap": _bf(_pswap16()),
        "eqb": _bf(_eb(qn_w)),
        "ekb": _bf(_eb(kn_w)),
        "wo": _bf(Wo.T),  # [c_in, c_out]
    }
    in_maps = []
    for c in range(N_CORES):
        b, g = c // 4, c % 4
        fs = slice(GF * g, GF * (g + 1))
        in_maps.append(
            dict(
                common,
                xt=_bf(x[b, :t, :].T),
                wq=_bf(_perm_wqk(Wq[fs, :].T)),
                wk=_bf(_perm_wqk(Wk[fs, :].T)),
                wv=_bf(Wv[fs, :].T),
            )
        )
    return in_maps


def assemble(results, t=T):
    tsl = t // 8
    out = np.empty((B, t, C), np.float32)
    for c in range(N_CORES):
        res = results[c]["out"]  # [C, 2*tsl]
        out[0, c * tsl : (c + 1) * tsl, :] = res[:, 0:tsl].T
        out[1, c * tsl : (c + 1) * tsl, :] = res[:, tsl : 2 * tsl].T
    return out


# -- cached PJRT runner (compile once, reuse across kernel() calls) ---------

_RUNNER = {}


def _get_runner(t=T):
    if t in _RUNNER:
        return _RUNNER[t]
    import jax
    from jax.sharding import Mesh, PartitionSpec
    from jax.experimental.shard_map import shard_map
    from concourse import bass2jax

    nc = build_nc(t)
    bass2jax.install_neuronx_cc_hook()

    partition_name = nc.partition_id_tensor.name if nc.partition_id_tensor else None
    in_names = []
    out_names = []
    out_avals = []
    zero_outs = []
    for alloc in nc.m.functions[0].allocations:
        if not isinstance(alloc, mybir.MemoryLocationSet):
            continue
        name = alloc.memorylocations[0].name
        if alloc.kind == "ExternalInput":
            if name == partition_name:
                continue
            in_names.append(name)
        elif alloc.kind == "ExternalOutput":
            shape = tuple(alloc.tensor_shape)
            dtype = mybir.dt.np(alloc.dtype)
            out_names.append(name)
            out_avals.append(jax.core.ShapedArray(shape, dtype))
            zero_outs.append(np.zeros(shape, dtype))
    n_params = len(in_names)
    all_names = in_names + out_names
    if partition_name is not None:
        all_names = all_names + [partition_name]

    def _body(*args):
        operands = list(args)
        if partition_name is not None:
            operands.append(bass2jax.partition_id_tensor())
        outs = bass2jax._bass_exec_p.bind(
            *operands,
            out_avals=tuple(out_avals),
            in_names=tuple(all_names),
            out_names=tuple(out_names),
            lowering_input_output_aliases=(),
            sim_require_finite=True,
            sim_require_nnan=True,
            nc=nc,
        )
        return tuple(outs)

    devices = jax.devices()[:N_CORES]
    mesh = Mesh(np.asarray(devices), ("core",))
    fn = jax.jit(
        shard_map(
            _body,
            mesh=mesh,
            in_specs=(PartitionSpec("core"),) * (n_params + len(out_names)),
            out_specs=(PartitionSpec("core"),) * len(out_names),
            check_rep=False,
        ),
        keep_unused=True,
    )
    runner = {
        "fn": fn,
        "body": _body,
        "in_names": in_names,
        "out_names": out_names,
        "out_avals": out_avals,
        "zero_outs": zero_outs,
        "jax": jax,
    }
    _RUNNER[t] = runner
    return runner


def run_device(in_maps, t=T):
    r = _get_runner(t)
    concat_in = [
        np.concatenate([np.asarray(m[name]) for m in in_maps], axis=0)
        for name in r["in_names"]
    ]
    concat_zero = [
        np.zeros((N_CORES * z.shape[0], *z.shape[1:]), z.dtype) for z in r["zero_outs"]
    ]
    outs = r["fn"](*concat_in, *concat_zero)
    results = []
    for c in range(N_CORES):
        results.append(
            {
                name: np.asarray(outs[i]).reshape(N_CORES, *r["out_avals"][i].shape)[c]
                for i, name in enumerate(r["out_names"])
            }
        )
    return results


def kernel(x, Wq, Wk, Wv, Wo, qn_w, kn_w):
    x = np.asarray(x, np.float32)
    in_maps = make_in_maps(
        x,
        np.asarray(Wq, np.float32),
        np.asarray(Wk, np.float32),
        np.asarray(Wv, np.float32),
        np.asarray(Wo, np.float32),
        np.asarray(qn_w, np.float32),
        np.asarray(kn_w, np.float32),
    )
    results = run_device(in_maps)
    return assemble(results)
